# revision 38
# baseline (speedup 1.0000x reference)
"""Additive (Bahdanau) attention kernel for 8 TRN2 NeuronCores.

Reference computation:
    q = queries @ Wq                      [B,Q,H]
    k = keys @ Wk                         [B,K,H]
    scores = einsum('bqkh,h->bqk', tanh(q[:,:,None,:] + k[:,None,:,:]), wv)
    out = softmax(scores, -1) @ values    [B,Q,V]

The naive form needs a [B,Q,K,H] tanh (134M ScalarE ops, ~110us/core).
Instead we expand tanh as a short sine series (tanh is odd):

    tanh(t) ~= sum_m beta_m * sin(omega_m * t)        max err 3.0e-3 on [-11,11]

and use the angle-addition identity to make the [Q,K] score map a pure
TensorEngine matmul:

    sum_h wv_h tanh(a_h + b_h)
      = sum_{m,h} [beta_m wv_h sin(om_m a_h)] * [cos(om_m b_h)]
      + sum_{m,h} [beta_m wv_h cos(om_m a_h)] * [sin(om_m b_h)]

i.e. scores = Fq @ Fk^T with 2*M*H = 512 feature rows per side.

Per core: project (float32r matmul, contraction dim pre-transposed on the
host), expand h -> (m,h) rows scaled by om_m/2pi via a tiny constant
matmul (so the sine arguments arrive in turns), range-reduce to
[-1/2, 1/2] turns (fs = x - round(x), fp32 magic-add; ScalarE's Sin spline
only covers [-pi, pi]), then sin = Sin(fs, scale=2pi) and - cosine being
even - cos = Sin(|fs|, scale=-2pi, bias=pi/2). Frequencies are snapped to
the fp16 grid (betas refit) so the fp16 expansion weights are exact.

Softmax over keys skips the max-subtraction (|scores| <= sum|wv_h| ~ 4.5,
exp is safe in fp32/fp16), and the denominator falls out of the PV matmul
via a ones-column appended to values.

Sharding: 8 shards = batch (4) x query-half (2); fully data-parallel, no
collectives.
"""

from contextlib import ExitStack

import numpy as np

import concourse.bass as bass
import concourse.tile as tile
from concourse import bacc, mybir
from concourse.bass_utils import run_bass_kernel_spmd
from concourse.tile_rust import add_dep_helper

# Problem shapes (hardcoded per the task statement).
B, Q, K = 4, 1024, 1024
E, H, V = 512, 32, 256
NCORES = 8
QC = Q // 2            # query rows per core

# Sine expansion of tanh on [-11, 11]; data range is |a+b| <= 8.8.
# Frequencies are stored as turns (omega/2pi) snapped to fp16; betas refit.
# Offline function-approximation constants, not data-derived.
OMEGA_TURNS = np.array([
    0.033355712890625, 0.10198974609375, 0.1746826171875,
    0.2509765625, 0.330078125, 0.411376953125,
    0.493896484375, 0.57568359375,
])
BETA = np.array([
    1.257769416928334, 0.3739298547081357, 0.17254946950332434,
    0.08281244397248748, 0.03896996975246528, 0.01786011049970509,
    0.00795846995302151, 0.00328524152579222,
])
M = len(OMEGA_TURNS)
MH = M * H             # pre-activation rows (sine arguments)
F = 2 * MH             # feature rows per side (sin + cos)

NE = E // 128          # contraction chunks for the projections
NPT = MH // 128        # pre-activation row tiles
NKT = K // 128         # key tiles
NQT = QC // 128        # query tiles
HALF = 512             # PSUM bank width in fp32
VA = V + 1             # values + denominator ones-column

F32 = mybir.dt.float32
F32R = mybir.dt.float32r
F16 = mybir.dt.float16
ACTF = mybir.ActivationFunctionType
ALU = mybir.AluOpType
PI_2 = float(np.pi / 2)
TWO_PI = float(2 * np.pi)
MAGIC = float(1.5 * 2 ** 23)   # fp32 round-to-nearest-integer magic constant


def _build_body(ctx, tc, aps):
    nc = tc.nc
    qT, kT, wbund, obund, vbund, out = aps

    const = ctx.enter_context(tc.tile_pool(name="const", bufs=1))
    feat = ctx.enter_context(tc.tile_pool(name="feat", bufs=1))
    tmp = ctx.enter_context(tc.tile_pool(name="tmp", bufs=4))
    pre_ps = ctx.enter_context(tc.tile_pool(name="pre_ps", bufs=2, space="PSUM"))
    sc_ps = ctx.enter_context(tc.tile_pool(name="sc_ps", bufs=2, space="PSUM"))
    pv_ps = ctx.enter_context(tc.tile_pool(name="pv_ps", bufs=1, space="PSUM"))

    # ---- PE warmup: the HAM clock-gate halves PE speed unless the array
    # has been continuously busy ~3us, so burn dummy matmuls through the
    # input-DMA window; the projections then start at full clock.
    warm = const.tile([128, 512], F16, name="warm")
    nc.vector.memset(warm[:], 0.5)

    def pe_trickle(n, cols=512):
        for _ in range(n):
            wps = sc_ps.tile([128, cols], F32, name="wps", tag="sc")
            nc.tensor.matmul(wps[:], warm[:, 0:128], warm[:, 0:cols],
                             start=True, stop=True)

    pe_trickle(12)

    # ---- stage inputs in SBUF (one DMA each, in consumption order) ----
    qT_sb = const.tile([128, NE * QC], F32R, name="qT_sb")
    qT3 = qT.rearrange("(c p) q -> p c q", p=128)
    for g in range(2):   # halves, so the first projection matmuls start early
        nc.sync.dma_start(
            qT_sb[:].rearrange("p (c q) -> p c q", c=NE)[:, 2 * g: 2 * g + 2],
            qT3[:, 2 * g: 2 * g + 2])
    wb_sb = const.tile([128, 2 * NE * H + NPT], F32R, name="wb_sb")
    nc.sync.dma_start(wb_sb[:], wbund[:, :])
    ob_sb = const.tile([128, MH], F16, name="ob_sb")
    nc.sync.dma_start(ob_sb[:], obund[:, :])
    kT_sb = const.tile([128, NE * K], F32R, name="kT_sb")
    kT3 = kT.rearrange("(c p) q -> p c q", p=128)
    kT4 = kT_sb[:].rearrange("p (h c q) -> p h c q", h=2, c=NE)
    for h in range(K // HALF):   # split so h=0 key features start earlier
        for g in range(2):
            nc.sync.dma_start(
                kT4[:, h, 2 * g: 2 * g + 2],
                kT3[:, 2 * g: 2 * g + 2, h * HALF:(h + 1) * HALF])
    vb_sb = const.tile([128, NKT * V], F32, name="vb_sb")
    nc.sync.dma_start(vb_sb[:], vbund[:, :])

    def wq_ap(e):
        return wb_sb[:, e * H: (e + 1) * H]

    def wk_ap(e):
        off = NE * H
        return wb_sb[:, off + e * H: off + (e + 1) * H]

    amp_off = 2 * NE * H
    half_pi = const.tile([128, 1], F32, name="half_pi")
    nc.vector.memset(half_pi[:], PI_2)

    # values + ones column, fp16: va_all viewed as [128, NKT, VA].
    # The value copy itself is emitted after the feature phase (see below)
    # so it cannot block the GpSimd-free engines mid-pipeline.
    va_all = const.tile([128, NKT * VA], F16, name="va_all")
    va3 = va_all[:].rearrange("p (t v) -> p t v", t=NKT)
    nc.gpsimd.memset(va3[:, :, V:VA], 1.0)

    # ---- projections: a = W^T x (fp32r), copied to fp16 for the expand ----
    a16_q = const.tile([32, QC], F16, name="a16_q")
    aps_q = sc_ps.tile([32, QC], F32, name="aps_q", tag="sc")
    for e in range(NE):
        nc.tensor.matmul(aps_q[:], wq_ap(e), qT_sb[:, bass.ts(e, QC)],
                         start=(e == 0), stop=(e == NE - 1))
    nc.vector.tensor_copy(a16_q[:], aps_q[:])

    a16_k = const.tile([32, K], F16, name="a16_k")
    for h in range(K // HALF):
        aps_k = sc_ps.tile([32, HALF], F32, name="aps_k", tag="sc")
        for e in range(NE):
            nc.tensor.matmul(
                aps_k[:], wk_ap(e),
                kT_sb[:, (h * NE + e) * HALF: (h * NE + e + 1) * HALF],
                start=(e == 0), stop=(e == NE - 1))
        nc.vector.tensor_copy(a16_k[:, bass.ts(h, HALF)], aps_k[:])

    # ---- feature generation ----
    # q side: qf16[2p] = amp * sin(pre_q[p]),  qf16[2p+1] = amp * cos(pre_q[p])
    # k side: kf16[2p] = cos(pre_k[p]),        kf16[2p+1] = sin(pre_k[p])
    qf16 = [feat.tile([128, QC], F16, name=f"qf{i}") for i in range(2 * NPT)]
    kf16 = [feat.tile([128, K], F16, name=f"kf{i}") for i in range(2 * NPT)]
    sin_acts = []
    fa_ops = []

    def gen_features(a16_src, p, width, sin_dst, cos_dst):
        """Expand h rows to (m,h)*om rows (turns), range-reduce, emit
        sin/cos fp16 feature tiles.

        fs = x - round(x) in [-1/2, 1/2]  (fp32 magic-add rounding)
        sin(y) = Sin(fs, scale=2pi)
        cos(y) = Sin(|fs|, scale=-2pi, bias=pi/2)   (cosine is even)
        """
        ps = pre_ps.tile([128, width], F32, name="pre", tag="pre")
        nc.tensor.matmul(ps[:], ob_sb[0:32, bass.ts(p, 128)], a16_src,
                         start=True, stop=True)
        rnd = tmp.tile([128, width], F32, name="rnd", tag=f"rnd{width}")
        nc.vector.tensor_scalar(rnd[:], ps[:], MAGIC, MAGIC, ALU.add, ALU.subtract)
        fs = tmp.tile([128, width], F16, name="fs", tag=f"fs{width}")
        nc.vector.tensor_tensor(fs[:], ps[:], rnd[:], ALU.subtract)
        fa = tmp.tile([128, width], F16, name="fa", tag=f"fa{width}")
        fi = nc.vector.scalar_tensor_tensor(fa[:], fs[:], -1.0, fs[:],
                                            ALU.mult, ALU.max)
        fa_ops.append(fi.ins)
        i1 = nc.scalar.activation(sin_dst, fs[:], ACTF.Sin, scale=TWO_PI)
        i2 = nc.scalar.activation(cos_dst, fa[:], ACTF.Sin, bias=half_pi[:, 0:1],
                                  scale=-TWO_PI)
        sin_acts.extend([i1.ins, i2.ins])

    for p in range(NPT):
        tsin = tmp.tile([128, QC], F16, name="qsin", tag="qsin")
        tcos = tmp.tile([128, QC], F16, name="qcos", tag="qcos")
        gen_features(a16_q[:], p, QC, tsin[:], tcos[:])
        amp_ap = wb_sb[:, amp_off + p: amp_off + p + 1].bitcast(F32)
        nc.vector.tensor_scalar_mul(qf16[2 * p][:], tsin[:], amp_ap)
        nc.vector.tensor_scalar_mul(qf16[2 * p + 1][:], tcos[:], amp_ap)

    for h in range(K // HALF):
        for p in range(NPT):
            gen_features(a16_k[:, bass.ts(h, HALF)], p, HALF,
                         kf16[2 * p + 1][:, bass.ts(h, HALF)],
                         kf16[2 * p][:, bass.ts(h, HALF)])

    vci = nc.gpsimd.tensor_copy(va3[:, :, 0:V], vb_sb[:, 0:NKT * V]
                                .rearrange("p (t v) -> p t v", t=NKT))
    for fo in fa_ops:   # keep the big copy out of the |fs| ops' way
        add_dep_helper(vci.ins, fo, sync=False, reason="va copy after fa ops")

    # ---- scores^T (pairing matmul) -> exp -> PV ----
    # All 8 score tiles get their own PSUM bank: the preact pool is idle by
    # the score phase, and the PV banks are only needed after exp(kt=0), so
    # scores kt4-7 borrow them (the pool WAW dep hands each bank to PV as
    # its exp drains). Without this, scores serialize behind the fenced exps.
    es16 = [feat.tile([128, QC], F16, name=f"es{kt}") for kt in range(NKT)]
    for kt in range(NKT):
        if kt < 2:
            ps = sc_ps.tile([128, QC], F32, name="sc", tag="sc")
        elif kt < 4:
            ps = pre_ps.tile([128, QC], F32, name="sc", tag="pre")
        else:
            ps = pv_ps.tile([128, QC], F32, name="sc", tag=f"pv{kt - 4}")
        for fc in range(2 * NPT):
            nc.tensor.matmul(ps[:], kf16[fc][:, bass.ts(kt, 128)], qf16[fc][:],
                             start=(fc == 0), stop=(fc == 2 * NPT - 1))
        ei = nc.scalar.activation(es16[kt][:], ps[:], ACTF.Exp)
        # keep every Exp after every Sin on ScalarE: each table-set switch
        # costs ~1.3us, and the scheduler would otherwise interleave them
        for si in sin_acts:
            add_dep_helper(ei.ins, si, sync=False, reason="act table set order")

    pv_tiles = [pv_ps.tile([128, VA], F32, name=f"pv{qt}", tag=f"pv{qt}")
                for qt in range(NQT)]
    for kt in range(NKT):
        for qt in range(NQT):
            nc.tensor.matmul(pv_tiles[qt][:],
                             es16[kt][:, bass.ts(qt, 128)],
                             va_all[:, kt * VA: (kt + 1) * VA],
                             start=(kt == 0), stop=(kt == NKT - 1))

    # ---- normalize and store (one output DMA) ----
    ot_all = const.tile([128, NQT * V], F32, name="ot_all")
    for qt in range(NQT):
        recip = tmp.tile([128, 1], F32, name="recip", tag="recip")
        nc.vector.reciprocal(recip[:], pv_tiles[qt][:, V:VA])
        nc.vector.tensor_scalar_mul(ot_all[:, bass.ts(qt, V)],
                                    pv_tiles[qt][:, 0:V], recip[:, 0:1])
    out3 = out.rearrange("(t p) v -> p t v", p=128)
    for g in range(2):
        gq = NQT // 2
        nc.sync.dma_start(out3[:, g * gq:(g + 1) * gq],
                          ot_all[:, g * gq * V:(g + 1) * gq * V]
                          .rearrange("p (t v) -> p t v", t=gq))



def build_nc():
    nc = bacc.Bacc(
        "TRN2",
        target_bir_lowering=False,
        debug=False,
        num_devices=NCORES,
    )
    qT = nc.dram_tensor("qT", [E, QC], F32R, kind="ExternalInput").ap()
    kT = nc.dram_tensor("kT", [E, K], F32R, kind="ExternalInput").ap()
    wbund = nc.dram_tensor("wbund", [128, 2 * NE * H + NPT], F32R,
                           kind="ExternalInput").ap()
    obund = nc.dram_tensor("obund", [128, MH], F16, kind="ExternalInput").ap()
    vbund = nc.dram_tensor("vbund", [128, NKT * V], F32,
                           kind="ExternalInput").ap()
    out = nc.dram_tensor("out", [QC, V], F32, kind="ExternalOutput").ap()
    with tile.TileContext(nc) as tc:
        with ExitStack() as ctx:
            _build_body(ctx, tc, (qT, kT, wbund, obund, vbund, out))
    nc.compile()
    return nc


def _tile_pack(x, p=128):
    """[C*p, N] -> [p, C*N] (row-chunk c lands at column block c)."""
    c = x.shape[0] // p
    return np.ascontiguousarray(
        x.reshape(c, p, x.shape[1]).transpose(1, 0, 2).reshape(p, -1))


def make_in_maps(queries, keys, values, Wq, Wk, wv):
    qf = np.asarray(queries, np.float32)
    kf = np.asarray(keys, np.float32)
    vf = np.asarray(values, np.float32)
    Wqf = np.asarray(Wq, np.float32)
    Wkf = np.asarray(Wk, np.float32)
    wvf = np.asarray(wv, np.float32)

    # amp[(m,h)] = beta[m] * wv[h], laid out [128, NPT]
    amp = (BETA.astype(np.float32)[:, None] * wvf[None, :]) \
        .reshape(MH).reshape(NPT, 128).T.astype(np.float32)
    wbund = np.concatenate([_tile_pack(Wqf), _tile_pack(Wkf), amp], axis=1)
    wbund = np.ascontiguousarray(wbund, np.float32)

    # obund[h, m*H+h] = om_m (turns), fp16-exact; zero-padded to 128 rows
    ob = np.zeros((128, MH), np.float16)
    for m, om in enumerate(OMEGA_TURNS):
        for h in range(H):
            ob[h, m * H + h] = np.float16(om)

    kT = [np.ascontiguousarray(kf[b].T) for b in range(B)]
    vbund = [np.ascontiguousarray(_tile_pack(vf[b]), np.float32)
             for b in range(B)]

    in_maps = []
    for core in range(NCORES):
        b, half = divmod(core, Q // QC)
        qT = np.ascontiguousarray(qf[b, half * QC:(half + 1) * QC].T)
        in_maps.append({
            "qT": qT,
            "kT": kT[b],
            "wbund": wbund,
            "obund": ob,
            "vbund": vbund[b],
        })
    return in_maps


_NC_CACHE = {}


def get_nc():
    if "nc" not in _NC_CACHE:
        _NC_CACHE["nc"] = build_nc()
    return _NC_CACHE["nc"]


def kernel(queries, keys, values, Wq, Wk, wv):
    nc = get_nc()
    in_maps = make_in_maps(queries, keys, values, Wq, Wk, wv)
    res = run_bass_kernel_spmd(nc, in_maps, core_ids=list(range(NCORES)))
    out = np.empty((B, Q, V), np.float32)
    for core in range(NCORES):
        b, half = divmod(core, Q // QC)
        out[b, half * QC:(half + 1) * QC] = res.results[core]["out"]
    return out



# revision 43
# speedup vs baseline: 1.0356x; 1.0356x over previous
"""Additive (Bahdanau) attention kernel for 8 TRN2 NeuronCores.

Reference computation:
    q = queries @ Wq                      [B,Q,H]
    k = keys @ Wk                         [B,K,H]
    scores = einsum('bqkh,h->bqk', tanh(q[:,:,None,:] + k[:,None,:,:]), wv)
    out = softmax(scores, -1) @ values    [B,Q,V]

The naive form needs a [B,Q,K,H] tanh (134M ScalarE ops, ~110us/core).
Instead we expand tanh as a short sine series (tanh is odd):

    tanh(t) ~= sum_m beta_m * sin(omega_m * t)        max err 3.0e-3 on [-11,11]

and use the angle-addition identity to make the [Q,K] score map a pure
TensorEngine matmul:

    sum_h wv_h tanh(a_h + b_h)
      = sum_{m,h} [beta_m wv_h sin(om_m a_h)] * [cos(om_m b_h)]
      + sum_{m,h} [beta_m wv_h cos(om_m a_h)] * [sin(om_m b_h)]

i.e. scores = Fq @ Fk^T with 2*M*H = 512 feature rows per side.

Per core: project (float32r matmul, contraction dim pre-transposed on the
host), expand h -> (m,h) rows scaled by om_m/2pi via a tiny constant
matmul (so the sine arguments arrive in turns), range-reduce to
[-1/2, 1/2] turns (fs = x - round(x), fp32 magic-add; ScalarE's Sin spline
only covers [-pi, pi]), then sin = Sin(fs, scale=2pi) and - cosine being
even - cos = Sin(|fs|, scale=-2pi, bias=pi/2). Frequencies are snapped to
the fp16 grid (betas refit) so the fp16 expansion weights are exact.

Softmax over keys skips the max-subtraction (|scores| <= sum|wv_h| ~ 4.5,
exp is safe in fp32/fp16), and the denominator falls out of the PV matmul
via a ones-column appended to values.

Sharding: 8 shards = batch (4) x query-half (2); fully data-parallel, no
collectives.
"""

from contextlib import ExitStack

import numpy as np

import concourse.bass as bass
import concourse.tile as tile
from concourse import bacc, mybir
from concourse.bass_utils import run_bass_kernel_spmd
from concourse.tile_rust import add_dep_helper

# Problem shapes (hardcoded per the task statement).
B, Q, K = 4, 1024, 1024
E, H, V = 512, 32, 256
NCORES = 8
QC = Q // 2            # query rows per core

# Sine expansion of tanh on [-11, 11]; data range is |a+b| <= 8.8.
# Frequencies are stored as turns (omega/2pi) snapped to fp16; betas refit.
# Offline function-approximation constants, not data-derived.
OMEGA_TURNS = np.array([
    0.033355712890625, 0.10198974609375, 0.1746826171875,
    0.2509765625, 0.330078125, 0.411376953125,
    0.493896484375, 0.57568359375,
])
BETA = np.array([
    1.257769416928334, 0.3739298547081357, 0.17254946950332434,
    0.08281244397248748, 0.03896996975246528, 0.01786011049970509,
    0.00795846995302151, 0.00328524152579222,
])
M = len(OMEGA_TURNS)
MH = M * H             # pre-activation rows (sine arguments)
F = 2 * MH             # feature rows per side (sin + cos)

NE = E // 128          # contraction chunks for the projections
NPT = MH // 128        # pre-activation row tiles
NKT = K // 128         # key tiles
NQT = QC // 128        # query tiles
HALF = 512             # PSUM bank width in fp32
VA = V + 1             # values + denominator ones-column

F32 = mybir.dt.float32
F32R = mybir.dt.float32r
F16 = mybir.dt.float16
ACTF = mybir.ActivationFunctionType
ALU = mybir.AluOpType
PI_2 = float(np.pi / 2)
TWO_PI = float(2 * np.pi)
MAGIC = float(1.5 * 2 ** 23)   # fp32 round-to-nearest-integer magic constant


def _build_body(ctx, tc, aps):
    nc = tc.nc
    qT, kT, wbund, obund, vbund, out = aps

    const = ctx.enter_context(tc.tile_pool(name="const", bufs=1))
    feat = ctx.enter_context(tc.tile_pool(name="feat", bufs=1))
    tmp = ctx.enter_context(tc.tile_pool(name="tmp", bufs=4))
    pre_ps = ctx.enter_context(tc.tile_pool(name="pre_ps", bufs=2, space="PSUM"))
    sc_ps = ctx.enter_context(tc.tile_pool(name="sc_ps", bufs=2, space="PSUM"))
    pv_ps = ctx.enter_context(tc.tile_pool(name="pv_ps", bufs=1, space="PSUM"))

    # ---- PE warmup: the HAM clock-gate halves PE speed unless the array
    # has been continuously busy ~3us, so burn dummy matmuls through the
    # input-DMA window; the projections then start at full clock.
    warm = const.tile([128, 512], F16, name="warm")
    nc.vector.memset(warm[:], 0.5)

    def pe_trickle(n, cols=512):
        for _ in range(n):
            wps = sc_ps.tile([128, cols], F32, name="wps", tag="sc")
            nc.tensor.matmul(wps[:], warm[:, 0:128], warm[:, 0:cols],
                             start=True, stop=True)

    pe_trickle(12)

    # ---- stage inputs in SBUF (one DMA each, in consumption order) ----
    qT_sb = const.tile([128, NE * QC], F32R, name="qT_sb")
    qT3 = qT.rearrange("(c p) q -> p c q", p=128)
    for g in range(2):   # halves, so the first projection matmuls start early
        nc.sync.dma_start(
            qT_sb[:].rearrange("p (c q) -> p c q", c=NE)[:, 2 * g: 2 * g + 2],
            qT3[:, 2 * g: 2 * g + 2])
    wb_sb = const.tile([128, 2 * NE * H + NPT], F32R, name="wb_sb")
    nc.sync.dma_start(wb_sb[:], wbund[:, :])
    ob_sb = const.tile([128, MH], F16, name="ob_sb")
    nc.sync.dma_start(ob_sb[:], obund[:, :])
    kT_sb = const.tile([128, NE * K], F32R, name="kT_sb")
    kT3 = kT.rearrange("(c p) q -> p c q", p=128)
    kT4 = kT_sb[:].rearrange("p (h c q) -> p h c q", h=2, c=NE)
    for h in range(K // HALF):   # split so h=0 key features start earlier
        for g in range(2):
            nc.sync.dma_start(
                kT4[:, h, 2 * g: 2 * g + 2],
                kT3[:, 2 * g: 2 * g + 2, h * HALF:(h + 1) * HALF])
    vb_sb = const.tile([128, NKT * V], F32, name="vb_sb")
    nc.sync.dma_start(vb_sb[:], vbund[:, :])

    def wq_ap(e):
        return wb_sb[:, e * H: (e + 1) * H]

    def wk_ap(e):
        off = NE * H
        return wb_sb[:, off + e * H: off + (e + 1) * H]

    amp_off = 2 * NE * H
    half_pi = const.tile([128, 1], F32, name="half_pi")
    nc.vector.memset(half_pi[:], PI_2)

    # values + ones column, fp16: va_all viewed as [128, NKT, VA].
    # The value copy itself is emitted after the feature phase (see below)
    # so it cannot block the GpSimd-free engines mid-pipeline.
    va_all = const.tile([128, NKT * VA], F16, name="va_all")
    va3 = va_all[:].rearrange("p (t v) -> p t v", t=NKT)
    nc.gpsimd.memset(va3[:, :, V:VA], 1.0)

    # ---- projections: a = W^T x (fp32r), copied to fp16 for the expand ----
    a16_q = const.tile([32, QC], F16, name="a16_q")
    aps_q = sc_ps.tile([32, QC], F32, name="aps_q", tag="sc")
    for e in range(NE):
        nc.tensor.matmul(aps_q[:], wq_ap(e), qT_sb[:, bass.ts(e, QC)],
                         start=(e == 0), stop=(e == NE - 1))
    nc.vector.tensor_copy(a16_q[:], aps_q[:])

    a16_k = const.tile([32, K], F16, name="a16_k")
    for h in range(K // HALF):
        aps_k = sc_ps.tile([32, HALF], F32, name="aps_k", tag="sc")
        for e in range(NE):
            nc.tensor.matmul(
                aps_k[:], wk_ap(e),
                kT_sb[:, (h * NE + e) * HALF: (h * NE + e + 1) * HALF],
                start=(e == 0), stop=(e == NE - 1))
        nc.vector.tensor_copy(a16_k[:, bass.ts(h, HALF)], aps_k[:])

    # ---- feature generation ----
    # q side: qf16[2p] = amp * sin(pre_q[p]),  qf16[2p+1] = amp * cos(pre_q[p])
    # k side: kf16[2p] = cos(pre_k[p]),        kf16[2p+1] = sin(pre_k[p])
    qf16 = [feat.tile([128, QC], F16, name=f"qf{i}") for i in range(2 * NPT)]
    kf16 = [feat.tile([128, K], F16, name=f"kf{i}") for i in range(2 * NPT)]
    sin_acts = []
    fa_ops = []

    def gen_features(a16_src, p, width, sin_dst, cos_dst):
        """Expand h rows to (m,h)*om rows (turns), range-reduce, emit
        sin/cos fp16 feature tiles.

        fs = x - round(x) in [-1/2, 1/2]  (fp32 magic-add rounding)
        sin(y) = Sin(fs, scale=2pi)
        cos(y) = Sin(|fs|, scale=-2pi, bias=pi/2)   (cosine is even)
        """
        ps = pre_ps.tile([128, width], F32, name="pre", tag="pre")
        nc.tensor.matmul(ps[:], ob_sb[0:32, bass.ts(p, 128)], a16_src,
                         start=True, stop=True)
        rnd = tmp.tile([128, width], F32, name="rnd", tag=f"rnd{width}")
        nc.vector.tensor_scalar(rnd[:], ps[:], MAGIC, MAGIC, ALU.add, ALU.subtract)
        fs = tmp.tile([128, width], F16, name="fs", tag=f"fs{width}")
        nc.vector.tensor_tensor(fs[:], ps[:], rnd[:], ALU.subtract)
        fa = tmp.tile([128, width], F16, name="fa", tag=f"fa{width}")
        fi = nc.vector.scalar_tensor_tensor(fa[:], fs[:], -1.0, fs[:],
                                            ALU.mult, ALU.max)
        fa_ops.append(fi.ins)
        i1 = nc.scalar.activation(sin_dst, fs[:], ACTF.Sin, scale=TWO_PI)
        i2 = nc.scalar.activation(cos_dst, fa[:], ACTF.Sin, bias=half_pi[:, 0:1],
                                  scale=-TWO_PI)
        sin_acts.extend([i1.ins, i2.ins])

    for p in range(NPT):
        tsin = tmp.tile([128, QC], F16, name="qsin", tag="qsin")
        tcos = tmp.tile([128, QC], F16, name="qcos", tag="qcos")
        gen_features(a16_q[:], p, QC, tsin[:], tcos[:])
        amp_ap = wb_sb[:, amp_off + p: amp_off + p + 1].bitcast(F32)
        nc.vector.tensor_scalar_mul(qf16[2 * p][:], tsin[:], amp_ap)
        nc.vector.tensor_scalar_mul(qf16[2 * p + 1][:], tcos[:], amp_ap)

    for h in range(K // HALF):
        for p in range(NPT):
            gen_features(a16_k[:, bass.ts(h, HALF)], p, HALF,
                         kf16[2 * p + 1][:, bass.ts(h, HALF)],
                         kf16[2 * p][:, bass.ts(h, HALF)])

    vci = nc.gpsimd.tensor_copy(va3[:, :, 0:V], vb_sb[:, 0:NKT * V]
                                .rearrange("p (t v) -> p t v", t=NKT))
    for fo in fa_ops:   # keep the big copy out of the |fs| ops' way
        add_dep_helper(vci.ins, fo, sync=False, reason="va copy after fa ops")

    # ---- scores^T (pairing matmul) -> exp -> PV ----
    # All 8 score tiles get their own PSUM bank: the preact pool is idle by
    # the score phase, and the PV banks are only needed after exp(kt=0), so
    # scores kt4-7 borrow them (the pool WAW dep hands each bank to PV as
    # its exp drains). Without this, scores serialize behind the fenced exps.
    es16 = [feat.tile([128, QC], F16, name=f"es{kt}") for kt in range(NKT)]
    for kt in range(NKT):
        if kt < 4:
            # kt0-3 borrow the PV banks: their exps drain first, handing
            # each bank to the PV accumulation as early as possible
            ps = pv_ps.tile([128, QC], F32, name="sc", tag=f"pv{kt}")
        elif kt < 6:
            ps = sc_ps.tile([128, QC], F32, name="sc", tag="sc")
        else:
            ps = pre_ps.tile([128, QC], F32, name="sc", tag="pre")
        for fc in range(2 * NPT):
            nc.tensor.matmul(ps[:], kf16[fc][:, bass.ts(kt, 128)], qf16[fc][:],
                             start=(fc == 0), stop=(fc == 2 * NPT - 1))
        ei = nc.scalar.activation(es16[kt][:], ps[:], ACTF.Exp)
        # keep every Exp after every Sin on ScalarE: each table-set switch
        # costs ~1.3us, and the scheduler would otherwise interleave them
        for si in sin_acts:
            add_dep_helper(ei.ins, si, sync=False, reason="act table set order")

    pv_tiles = [pv_ps.tile([128, VA], F32, name=f"pv{qt}", tag=f"pv{qt}")
                for qt in range(NQT)]
    for kt in range(NKT):
        for qt in range(NQT):
            nc.tensor.matmul(pv_tiles[qt][:],
                             es16[kt][:, bass.ts(qt, 128)],
                             va_all[:, kt * VA: (kt + 1) * VA],
                             start=(kt == 0), stop=(kt == NKT - 1))

    # ---- normalize and store (one output DMA) ----
    ot_all = const.tile([128, NQT * V], F32, name="ot_all")
    for qt in range(NQT):
        recip = tmp.tile([128, 1], F32, name="recip", tag="recip")
        nc.vector.reciprocal(recip[:], pv_tiles[qt][:, V:VA])
        nc.vector.tensor_scalar_mul(ot_all[:, bass.ts(qt, V)],
                                    pv_tiles[qt][:, 0:V], recip[:, 0:1])
    out3 = out.rearrange("(t p) v -> p t v", p=128)
    for g in range(2):
        gq = NQT // 2
        nc.sync.dma_start(out3[:, g * gq:(g + 1) * gq],
                          ot_all[:, g * gq * V:(g + 1) * gq * V]
                          .rearrange("p (t v) -> p t v", t=gq))



def build_nc():
    nc = bacc.Bacc(
        "TRN2",
        target_bir_lowering=False,
        debug=False,
        num_devices=NCORES,
    )
    qT = nc.dram_tensor("qT", [E, QC], F32R, kind="ExternalInput").ap()
    kT = nc.dram_tensor("kT", [E, K], F32R, kind="ExternalInput").ap()
    wbund = nc.dram_tensor("wbund", [128, 2 * NE * H + NPT], F32R,
                           kind="ExternalInput").ap()
    obund = nc.dram_tensor("obund", [128, MH], F16, kind="ExternalInput").ap()
    vbund = nc.dram_tensor("vbund", [128, NKT * V], F32,
                           kind="ExternalInput").ap()
    out = nc.dram_tensor("out", [QC, V], F32, kind="ExternalOutput").ap()
    with tile.TileContext(nc) as tc:
        with ExitStack() as ctx:
            _build_body(ctx, tc, (qT, kT, wbund, obund, vbund, out))
    nc.compile()
    return nc


def _tile_pack(x, p=128):
    """[C*p, N] -> [p, C*N] (row-chunk c lands at column block c)."""
    c = x.shape[0] // p
    return np.ascontiguousarray(
        x.reshape(c, p, x.shape[1]).transpose(1, 0, 2).reshape(p, -1))


def make_in_maps(queries, keys, values, Wq, Wk, wv):
    qf = np.asarray(queries, np.float32)
    kf = np.asarray(keys, np.float32)
    vf = np.asarray(values, np.float32)
    Wqf = np.asarray(Wq, np.float32)
    Wkf = np.asarray(Wk, np.float32)
    wvf = np.asarray(wv, np.float32)

    # amp[(m,h)] = beta[m] * wv[h], laid out [128, NPT]
    amp = (BETA.astype(np.float32)[:, None] * wvf[None, :]) \
        .reshape(MH).reshape(NPT, 128).T.astype(np.float32)
    wbund = np.concatenate([_tile_pack(Wqf), _tile_pack(Wkf), amp], axis=1)
    wbund = np.ascontiguousarray(wbund, np.float32)

    # obund[h, m*H+h] = om_m (turns), fp16-exact; zero-padded to 128 rows
    ob = np.zeros((128, MH), np.float16)
    for m, om in enumerate(OMEGA_TURNS):
        for h in range(H):
            ob[h, m * H + h] = np.float16(om)

    kT = [np.ascontiguousarray(kf[b].T) for b in range(B)]
    vbund = [np.ascontiguousarray(_tile_pack(vf[b]), np.float32)
             for b in range(B)]

    in_maps = []
    for core in range(NCORES):
        b, half = divmod(core, Q // QC)
        qT = np.ascontiguousarray(qf[b, half * QC:(half + 1) * QC].T)
        in_maps.append({
            "qT": qT,
            "kT": kT[b],
            "wbund": wbund,
            "obund": ob,
            "vbund": vbund[b],
        })
    return in_maps


_NC_CACHE = {}


def get_nc():
    if "nc" not in _NC_CACHE:
        _NC_CACHE["nc"] = build_nc()
    return _NC_CACHE["nc"]


def kernel(queries, keys, values, Wq, Wk, wv):
    nc = get_nc()
    in_maps = make_in_maps(queries, keys, values, Wq, Wk, wv)
    res = run_bass_kernel_spmd(nc, in_maps, core_ids=list(range(NCORES)))
    out = np.empty((B, Q, V), np.float32)
    for core in range(NCORES):
        b, half = divmod(core, Q // QC)
        out[b, half * QC:(half + 1) * QC] = res.results[core]["out"]
    return out

